# revision 15
# baseline (speedup 1.0000x reference)
"""Trainium2 Bass kernel for nn_GNN_82171314307289 (2x SAGEConv + GAT + pool + MLP).

8-core graph-parallel. Key ideas:
  - Nodes partitioned by owner core (8 graphs/core via sorted `batch`),
    degree-sorted within core, padded to NPAD/core. All index prep on host.
  - Aggregations gather rows of LINEAR tables (y = x @ W.T computed first, so
    gathered rows are 512B+ and the mean/attention combine happens after),
    via indirect_dma_start in its [128,1]-offset form (one 128-row gather per
    (node-tile, slot)); K-slot layout with degree-sorted tiles minimizes slots.
  - Per-core tables are laid out in ROTATED shard order (slab j = core
    (c+j)%8) so the SPMD program is core-independent (own shard = slab 0).
  - GAT: a_src/a_dst folded into the gather table as extra weight columns;
    self-loop handled locally from the own-row load; softmax + weighted sum on
    DVE/ACT per tile; g_b and the 0.5 head-mean folded downstream.
  - 4 launches (x1 | x2 | GAT+pool | decoder); host relays activation shards
    between launches (pure data movement; all flops on device).
"""
import math
import os
import numpy as np

ABL = set(os.environ.get("ABL", "").split(","))

import concourse.bass as bass
import concourse.bacc as bacc
import concourse.tile as tile
import concourse.mybir as mybir
from concourse.bass_utils import run_bass_kernel_spmd

NC = 8
P = 128
B = 64
EPS = 1e-5
F32 = mybir.dt.float32
I32 = mybir.dt.int32
AF = mybir.ActivationFunctionType
OP = mybir.AluOpType
XW_W = 260          # xw table row: 256 xw | 2 a_src | 2 a_dst

_programs = {}
LAST = {}


# ------------------------------------------------------------------ host prep

def _prep(pos, edge_index, batch):
    N = pos.shape[0]
    src = np.asarray(edge_index[0], dtype=np.int64)
    dst = np.asarray(edge_index[1], dtype=np.int64)
    batch = np.asarray(batch, dtype=np.int64)

    owner = batch // (B // NC)
    deg = np.bincount(dst, minlength=N)
    counts = np.bincount(owner, minlength=NC)
    NPAD = int(math.ceil((counts.max() + 16) / 128.0) * 128)
    T = NPAD // P
    NT = NC * NPAD
    ZR = NPAD - 1                     # guaranteed pad node -> all-zero table row

    # permutation: per core, degree-descending
    new_of_old = np.empty(N, dtype=np.int64)
    for c in range(NC):
        nodes = np.where(owner == c)[0]
        order = nodes[np.argsort(-deg[nodes], kind="stable")]
        new_of_old[order] = c * NPAD + np.arange(order.shape[0])

    deg_n = np.zeros(NC * NPAD, dtype=np.int64)
    deg_n[new_of_old] = deg
    batch_n = np.full(NC * NPAD, B, dtype=np.int64)
    batch_n[new_of_old] = batch
    real_n = np.zeros(NC * NPAD, dtype=np.float32)
    real_n[new_of_old] = 1.0

    src_n = new_of_old[src]
    dst_n = new_of_old[dst]
    c_dst = dst_n // NPAD
    dl = dst_n % NPAD

    # K schedule (shared across cores)
    K_tile = deg_n.reshape(NC, T, P).max(axis=2)          # [NC, T]
    K_sched = K_tile.max(axis=0).astype(np.int64)         # [T]
    SC = int(K_sched.sum())
    coff = np.zeros(T + 1, dtype=np.int64)
    coff[1:] = np.cumsum(K_sched)

    # slot fill: edges sorted by (core, dst, id) -> slot index per dst
    eorder = np.lexsort((np.arange(src.shape[0]), dst_n))
    ds = dst_n[eorder]
    ss = src_n[eorder]
    first = np.ones(ds.shape[0], dtype=bool)
    first[1:] = ds[1:] != ds[:-1]
    gi = np.arange(ds.shape[0])
    run0 = np.maximum.accumulate(np.where(first, gi, 0))
    slot = gi - run0

    # per-core idx + mask arrays [128, SC]
    idx_all, mask_all = [], []
    s_core = ss // NPAD
    s_loc = ss % NPAD
    d_core = ds // NPAD
    d_loc = ds % NPAD
    d_t = d_loc // P
    d_p = d_loc % P
    for c in range(NC):
        idx = np.full((P, SC), ZR, dtype=np.int64)        # pad -> zero row slab0
        msk = np.zeros((P, SC), dtype=np.float32)
        m = d_core == c
        j = (s_core[m] - c) % NC
        rot = j * NPAD + s_loc[m]
        col = coff[d_t[m]] + slot[m]
        idx[d_p[m], col] = rot
        msk[d_p[m], col] = 1.0
        idx_all.append(idx.astype(np.int32))
        mask_all.append(msk)

    def tilewise(arr, dtype=np.float32):
        return arr.reshape(NC, T, P).transpose(0, 2, 1).astype(dtype)  # [NC,128,T]

    invd = tilewise(1.0 / np.maximum(deg_n.astype(np.float64), 1.0))
    invdsl = tilewise(0.5 / (deg_n.astype(np.float64) + 1.0))
    batchf = tilewise(batch_n.astype(np.float64))
    realm = tilewise(real_n.astype(np.float64))

    cnt_graph = np.bincount(batch, minlength=B).astype(np.float64)
    inv_cnt = (1.0 / np.maximum(cnt_graph, 1.0)).astype(np.float32)

    meta = dict(N=N, NPAD=NPAD, T=T, NT=NT, SC=SC,
                K_sched=K_sched.tolist(), coff=coff.tolist(),
                new_of_old=new_of_old, inv_cnt=inv_cnt)
    percore = [dict(idx=idx_all[c], mask=mask_all[c], invd=invd[c],
                    invdsl=invdsl[c], batchf=batchf[c], realm=realm[c])
               for c in range(NC)]
    return meta, percore


def _rotate(full, c, NPAD, axis=1):
    """Rotate shards along `axis` so own (core c) is slab 0."""
    k = c * NPAD
    if axis == 0:
        return np.concatenate([full[k:], full[:k]], axis=0)
    return np.concatenate([full[:, k:], full[:, :k]], axis=1)


# ------------------------------------------------------------- bass programs

def _common_build(nc, meta, idx_name="idx", with_mask=False):
    pass


def _build_L1(meta):
    NPAD, T, NT, SC = meta['NPAD'], meta['T'], meta['NT'], meta['SC']
    K_sched, coff = meta['K_sched'], meta['coff']
    nc = bacc.Bacc("TRN2", target_bir_lowering=False, debug=False, num_devices=NC)
    pos4r = nc.dram_tensor("pos4r", [NT, 4], F32, kind="ExternalInput")
    pos4o = nc.dram_tensor("pos4o", [4, NPAD], F32, kind="ExternalInput")
    w1l = nc.dram_tensor("w1l", [4, P], F32, kind="ExternalInput")
    w1r = nc.dram_tensor("w1r", [4, P], F32, kind="ExternalInput")
    idx = nc.dram_tensor("idx", [P, SC], I32, kind="ExternalInput")
    invd = nc.dram_tensor("invd", [P, T], F32, kind="ExternalInput")
    realm = nc.dram_tensor("realm", [P, T], F32, kind="ExternalInput")
    ident = nc.dram_tensor("ident", [P, P], F32, kind="ExternalInput")
    x1T = nc.dram_tensor("x1T", [P, NPAD], F32, kind="ExternalOutput")
    x1R = nc.dram_tensor("x1R", [NPAD, P], F32, kind="ExternalOutput")

    with tile.TileContext(nc) as tc:
        with tc.tile_pool(name="const", bufs=1) as cp:
            w1l_s = cp.tile([4, P], F32)
            w1r_s = cp.tile([4, P], F32)
            p4o_s = cp.tile([4, NPAD], F32)
            id_s = cp.tile([P, P], F32)
            invd_s = cp.tile([P, T], F32)
            realm_s = cp.tile([P, T], F32)
            idx_s = cp.tile([P, SC], I32)
            nc.sync.dma_start(out=w1l_s[:], in_=w1l[:])
            nc.sync.dma_start(out=w1r_s[:], in_=w1r[:])
            nc.sync.dma_start(out=p4o_s[:], in_=pos4o[:])
            nc.sync.dma_start(out=id_s[:], in_=ident[:])
            nc.sync.dma_start(out=invd_s[:], in_=invd[:])
            nc.sync.dma_start(out=realm_s[:], in_=realm[:])
            nc.sync.dma_start(out=idx_s[:], in_=idx[:])

            with tc.tile_pool(name="pb", bufs=4) as pb, \
                 tc.tile_pool(name="pbp", bufs=3, space="PSUM") as pbp, \
                 tc.tile_pool(name="pbt", bufs=2, space="PSUM") as pbt, \
                 tc.tile_pool(name="pba", bufs=2, space="PSUM") as pba:
                for t in range(T):
                    K = K_sched[t]
                    rp = pbp.tile([P, P], F32, space="PSUM", tag="r")
                    if K > 0:
                        G = pb.tile([P, max(K, 2), 4], F32, tag="G")
                        for k in range(K):
                            nc.gpsimd.indirect_dma_start(
                                out=G[:, k, :], out_offset=None, in_=pos4r[:],
                                in_offset=bass.IndirectOffsetOnAxis(
                                    ap=idx_s[:, coff[t] + k:coff[t] + k + 1],
                                    axis=0))
                        acc = pb.tile([P, 4], F32, tag="acc")
                        if K == 1:
                            nc.vector.tensor_copy(out=acc[:], in_=G[:, 0, :])
                        else:
                            nc.vector.tensor_tensor(out=acc[:], in0=G[:, 0, :],
                                                    in1=G[:, 1, :], op=OP.add)
                            for k in range(2, K):
                                nc.vector.tensor_tensor(out=acc[:], in0=acc[:],
                                                        in1=G[:, k, :], op=OP.add)
                        nc.vector.tensor_scalar(
                            out=acc[:], in0=acc[:], scalar1=invd_s[:, t:t + 1],
                            scalar2=None, op0=OP.mult)
                        ta = pba.tile([4, P], F32, space="PSUM", tag="ta")
                        nc.tensor.transpose(out=ta[:], in_=acc[:], identity=id_s[:])
                        aggT = pb.tile([4, P], F32, tag="aggT")
                        nc.scalar.copy(out=aggT[:], in_=ta[:])
                        nc.tensor.matmul(rp[:], lhsT=aggT[:], rhs=w1l_s[:],
                                         start=True, stop=False)
                        nc.tensor.matmul(rp[:], lhsT=p4o_s[:, t * P:(t + 1) * P],
                                         rhs=w1r_s[:], start=False, stop=True)
                    else:
                        nc.tensor.matmul(rp[:], lhsT=p4o_s[:, t * P:(t + 1) * P],
                                         rhs=w1r_s[:], start=True, stop=True)
                    x1t = pb.tile([P, P], F32, tag="x1t")
                    nc.vector.tensor_scalar(
                        out=x1t[:], in0=rp[:], scalar1=0.0,
                        scalar2=realm_s[:, t:t + 1], op0=OP.max, op1=OP.mult)
                    nc.sync.dma_start(out=x1R[t * P:(t + 1) * P, :], in_=x1t[:])
                    tp = pbt.tile([P, P], F32, space="PSUM", tag="tr")
                    nc.tensor.transpose(out=tp[:], in_=x1t[:], identity=id_s[:])
                    xo = pb.tile([P, P], F32, tag="xo")
                    nc.scalar.copy(out=xo[:], in_=tp[:])
                    nc.sync.dma_start(out=x1T[:, t * P:(t + 1) * P], in_=xo[:])
    nc.compile()
    return nc


def _build_L2(meta):
    NPAD, T, NT, SC = meta['NPAD'], meta['T'], meta['NT'], meta['SC']
    K_sched, coff = meta['K_sched'], meta['coff']
    nc = bacc.Bacc("TRN2", target_bir_lowering=False, debug=False, num_devices=NC)
    x1r = nc.dram_tensor("x1r", [NT, P], F32, kind="ExternalInput")
    x1To = nc.dram_tensor("x1To", [P, NPAD], F32, kind="ExternalInput")
    w2l = nc.dram_tensor("w2l", [P, P], F32, kind="ExternalInput")
    w2r = nc.dram_tensor("w2r", [P, P], F32, kind="ExternalInput")
    s2b = nc.dram_tensor("s2b", [1, P], F32, kind="ExternalInput")
    idx = nc.dram_tensor("idx", [P, SC], I32, kind="ExternalInput")
    invd = nc.dram_tensor("invd", [P, T], F32, kind="ExternalInput")
    realm = nc.dram_tensor("realm", [P, T], F32, kind="ExternalInput")
    ident = nc.dram_tensor("ident", [P, P], F32, kind="ExternalInput")
    x2T = nc.dram_tensor("x2T", [P, NPAD], F32, kind="ExternalOutput")

    with tile.TileContext(nc) as tc:
        with tc.tile_pool(name="const", bufs=1) as cp:
            w2l_s = cp.tile([P, P], F32)
            w2r_s = cp.tile([P, P], F32)
            s2b_s = cp.tile([1, P], F32)
            ones_s = cp.tile([1, P], F32)
            id_s = cp.tile([P, P], F32)
            invd_s = cp.tile([P, T], F32)
            realm_s = cp.tile([P, T], F32)
            idx_s = cp.tile([P, SC], I32)
            nc.sync.dma_start(out=w2l_s[:], in_=w2l[:])
            nc.sync.dma_start(out=w2r_s[:], in_=w2r[:])
            nc.sync.dma_start(out=s2b_s[:], in_=s2b[:])
            nc.vector.memset(ones_s[:], 1.0)
            nc.sync.dma_start(out=id_s[:], in_=ident[:])
            nc.sync.dma_start(out=invd_s[:], in_=invd[:])
            nc.sync.dma_start(out=realm_s[:], in_=realm[:])
            nc.sync.dma_start(out=idx_s[:], in_=idx[:])

            with tc.tile_pool(name="pb", bufs=4) as pb, \
                 tc.tile_pool(name="pbp", bufs=3, space="PSUM") as pbp, \
                 tc.tile_pool(name="pbt", bufs=2, space="PSUM") as pbt, \
                 tc.tile_pool(name="pba", bufs=2, space="PSUM") as pba:
                for t in range(T):
                    K = K_sched[t]
                    xl = pb.tile([P, P], F32, tag="xl")
                    nc.sync.dma_start(out=xl[:], in_=x1To[:, t * P:(t + 1) * P])
                    rp = pbp.tile([P, P], F32, space="PSUM", tag="r")
                    nc.tensor.matmul(rp[:], lhsT=xl[:], rhs=w2r_s[:],
                                     start=True, stop=False)
                    nc.tensor.matmul(rp[:], lhsT=ones_s[:], rhs=s2b_s[:],
                                     start=False, stop=(K == 0))
                    if K > 0:
                        G = pb.tile([P, max(K, 2), P], F32, tag="G")
                        for k in range(K):
                            nc.gpsimd.indirect_dma_start(
                                out=G[:, k, :], out_offset=None, in_=x1r[:],
                                in_offset=bass.IndirectOffsetOnAxis(
                                    ap=idx_s[:, coff[t] + k:coff[t] + k + 1],
                                    axis=0))
                        acc = pb.tile([P, P], F32, tag="acc")
                        if K == 1:
                            nc.vector.tensor_copy(out=acc[:], in_=G[:, 0, :])
                        else:
                            nc.vector.tensor_tensor(out=acc[:], in0=G[:, 0, :],
                                                    in1=G[:, 1, :], op=OP.add)
                            for k in range(2, K):
                                nc.vector.tensor_tensor(out=acc[:], in0=acc[:],
                                                        in1=G[:, k, :], op=OP.add)
                        nc.vector.tensor_scalar(
                            out=acc[:], in0=acc[:], scalar1=invd_s[:, t:t + 1],
                            scalar2=None, op0=OP.mult)
                        ta = pba.tile([P, P], F32, space="PSUM", tag="ta")
                        nc.tensor.transpose(out=ta[:], in_=acc[:], identity=id_s[:])
                        aggT = pb.tile([P, P], F32, tag="aggT")
                        nc.scalar.copy(out=aggT[:], in_=ta[:])
                        nc.tensor.matmul(rp[:], lhsT=aggT[:], rhs=w2l_s[:],
                                         start=False, stop=True)
                    x2t = pb.tile([P, P], F32, tag="x2t")
                    nc.vector.tensor_scalar(
                        out=x2t[:], in0=rp[:], scalar1=0.0,
                        scalar2=realm_s[:, t:t + 1], op0=OP.max, op1=OP.mult)
                    tp = pbt.tile([P, P], F32, space="PSUM", tag="tr")
                    nc.tensor.transpose(out=tp[:], in_=x2t[:], identity=id_s[:])
                    xo = pb.tile([P, P], F32, tag="xo")
                    nc.scalar.copy(out=xo[:], in_=tp[:])
                    nc.sync.dma_start(out=x2T[:, t * P:(t + 1) * P], in_=xo[:])
    nc.compile()
    return nc


def _build_L3(meta):
    NPAD, T, NT, SC = meta['NPAD'], meta['T'], meta['NT'], meta['SC']
    K_sched, coff = meta['K_sched'], meta['coff']
    W = XW_W
    nc = bacc.Bacc("TRN2", target_bir_lowering=False, debug=False, num_devices=NC)
    x1T = nc.dram_tensor("x1T", [P, NT], F32, kind="ExternalInput")
    x2T = nc.dram_tensor("x2T", [P, NT], F32, kind="ExternalInput")
    w3a = nc.dram_tensor("w3a", [P, W], F32, kind="ExternalInput")
    w3b = nc.dram_tensor("w3b", [P, W], F32, kind="ExternalInput")
    idx = nc.dram_tensor("idx", [P, SC], I32, kind="ExternalInput")
    mask = nc.dram_tensor("mask", [P, SC], F32, kind="ExternalInput")
    invdsl = nc.dram_tensor("invdsl", [P, T], F32, kind="ExternalInput")
    batchf = nc.dram_tensor("batchf", [P, T], F32, kind="ExternalInput")
    iota64 = nc.dram_tensor("iota64", [P, B], F32, kind="ExternalInput")
    zT = nc.dram_tensor("zT", [P, B], F32, kind="ExternalOutput")
    xw = nc.dram_tensor("xwtab", [NT, W], F32, kind="Internal")

    with tile.TileContext(nc) as tc:
        with tc.tile_pool(name="const", bufs=1) as cp:
            w3a_s = cp.tile([P, W], F32)
            w3b_s = cp.tile([P, W], F32)
            invdsl_s = cp.tile([P, T], F32)
            batchf_s = cp.tile([P, T], F32)
            iota_s = cp.tile([P, B], F32)
            idx_s = cp.tile([P, SC], I32)
            mask_s = cp.tile([P, SC], F32)
            nc.sync.dma_start(out=w3a_s[:], in_=w3a[:])
            nc.sync.dma_start(out=w3b_s[:], in_=w3b[:])
            nc.sync.dma_start(out=invdsl_s[:], in_=invdsl[:])
            nc.sync.dma_start(out=batchf_s[:], in_=batchf[:])
            nc.sync.dma_start(out=iota_s[:], in_=iota64[:])
            nc.sync.dma_start(out=idx_s[:], in_=idx[:])
            nc.sync.dma_start(out=mask_s[:], in_=mask[:])

            # phase A: xw table
            with tc.tile_pool(name="pa", bufs=3) as pa, \
                 tc.tile_pool(name="pap", bufs=4, space="PSUM") as pap:
                for g in range(NT // 512):
                    xs1 = pa.tile([P, 512], F32, tag="xs1")
                    xs2 = pa.tile([P, 512], F32, tag="xs2")
                    nc.sync.dma_start(out=xs1[:], in_=x1T[:, g * 512:(g + 1) * 512])
                    nc.sync.dma_start(out=xs2[:], in_=x2T[:, g * 512:(g + 1) * 512])
                    for u in range(4):
                        wp = pap.tile([P, W], F32, space="PSUM")
                        nc.tensor.matmul(wp[:], lhsT=xs1[:, u * P:(u + 1) * P],
                                         rhs=w3a_s[:], start=True, stop=False)
                        nc.tensor.matmul(wp[:], lhsT=xs2[:, u * P:(u + 1) * P],
                                         rhs=w3b_s[:], start=False, stop=True)
                        ws = pa.tile([P, W], F32, tag="ws")
                        nc.vector.tensor_copy(out=ws[:], in_=wp[:])
                        nc.sync.dma_start(
                            out=xw[g * 512 + u * P:g * 512 + (u + 1) * P, :],
                            in_=ws[:])

            # phase B: GAT per tile + pooling
            with tc.tile_pool(name="pb", bufs=3) as pb, \
                 tc.tile_pool(name="pz", bufs=1, space="PSUM") as pz:
                zp = pz.tile([P, B], F32, space="PSUM")
                for t in range(T):
                    K = K_sched[t]
                    Lt = pb.tile([P, W], F32, tag="Lt")
                    nc.sync.dma_start(out=Lt[:], in_=xw[t * P:(t + 1) * P, :])
                    # self attention logit
                    es = pb.tile([P, 2], F32, tag="es")
                    nc.vector.tensor_tensor(out=es[:], in0=Lt[:, 256:258],
                                            in1=Lt[:, 258:260], op=OP.add)
                    # leaky_relu(x,0.2) = max(0.2x, x); HW Lrelu ignores alpha
                    nc.vector.scalar_tensor_tensor(
                        out=es[:], in0=es[:], scalar=0.2, op0=OP.mult,
                        in1=es[:], op1=OP.max)
                    acc3 = pb.tile([P, 2, P], F32, tag="acc3")
                    if K > 0:
                        G = pb.tile([P, max(K, 2), W], F32, tag="G")
                        for k in range(K):
                            if "nogather" in ABL:
                                break
                            nc.gpsimd.indirect_dma_start(
                                out=G[:, k, :], out_offset=None, in_=xw[:],
                                in_offset=bass.IndirectOffsetOnAxis(
                                    ap=idx_s[:, coff[t] + k:coff[t] + k + 1],
                                    axis=0))
                        et = pb.tile([P, 2, max(K, 2)], F32, tag="et")
                        nc.vector.tensor_tensor(
                            out=et[:, :, :K],
                            in0=G[:, :K, 256:258].rearrange("p k h -> p h k"),
                            in1=Lt[:, 258:260][:, :, None].to_broadcast([P, 2, K]),
                            op=OP.add)
                        nc.vector.scalar_tensor_tensor(
                            out=et[:, :, :K], in0=et[:, :, :K], scalar=0.2,
                            op0=OP.mult, in1=et[:, :, :K], op1=OP.max)
                        mx = pb.tile([P, 2], F32, tag="mx")
                        nc.vector.tensor_reduce(out=mx[:], in_=et[:, :, :K],
                                                axis=mybir.AxisListType.X,
                                                op=OP.max)
                        nc.vector.tensor_tensor(out=mx[:], in0=mx[:], in1=es[:],
                                                op=OP.max)
                        ngm = pb.tile([P, 2], F32, tag="ngm")
                        nc.vector.tensor_scalar(out=ngm[:], in0=mx[:],
                                                scalar1=-1.0, scalar2=None,
                                                op0=OP.mult)
                        pt = pb.tile([P, 2, max(K, 2)], F32, tag="pt")
                        for h in range(2):
                            nc.scalar.activation(pt[:, h, :K], et[:, h, :K],
                                                 AF.Exp, bias=ngm[:, h:h + 1])
                        nc.vector.tensor_tensor(
                            out=pt[:, :, :K], in0=pt[:, :, :K],
                            in1=mask_s[:, coff[t]:coff[t] + K][:, None, :]
                            .to_broadcast([P, 2, K]),
                            op=OP.mult)
                        # self prob
                        pself = pb.tile([P, 2], F32, tag="pself")
                        nc.vector.tensor_tensor(out=pself[:], in0=es[:],
                                                in1=mx[:], op=OP.subtract)
                        nc.scalar.activation(pself[:], pself[:], AF.Exp)
                        dn = pb.tile([P, 2], F32, tag="dn")
                        nc.vector.tensor_reduce(out=dn[:], in_=pt[:, :, :K],
                                                axis=mybir.AxisListType.X,
                                                op=OP.add)
                        nc.vector.tensor_tensor(out=dn[:], in0=dn[:],
                                                in1=pself[:], op=OP.add)
                        rd = pb.tile([P, 2], F32, tag="rd")
                        nc.vector.reciprocal(rd[:], dn[:])
                        nc.vector.tensor_scalar(out=rd[:], in0=rd[:],
                                                scalar1=invdsl_s[:, t:t + 1],
                                                scalar2=None, op0=OP.mult)
                        # weighted messages
                        tm = pb.tile([P, max(K, 2), 2, P], F32, tag="tm")
                        nc.vector.tensor_tensor(
                            out=tm[:, :K], in0=G[:, :K, 0:256]
                            .rearrange("p k (h c) -> p k h c", h=2),
                            in1=pt[:].rearrange("p h k -> p k h")[:, :K, :, None]
                            .to_broadcast([P, K, 2, P]),
                            op=OP.mult)
                        if K == 1:
                            nc.vector.tensor_copy(out=acc3[:], in_=tm[:, 0])
                        else:
                            nc.vector.tensor_tensor(out=acc3[:], in0=tm[:, 0],
                                                    in1=tm[:, 1], op=OP.add)
                            for k in range(2, K):
                                nc.vector.tensor_tensor(out=acc3[:], in0=acc3[:],
                                                        in1=tm[:, k], op=OP.add)
                    else:
                        pself = pb.tile([P, 2], F32, tag="pself")
                        nc.vector.memset(pself[:], 1.0)
                        rd = pb.tile([P, 2], F32, tag="rd")
                        nc.vector.tensor_scalar(out=rd[:], in0=pself[:],
                                                scalar1=invdsl_s[:, t:t + 1],
                                                scalar2=None, op0=OP.mult)
                        nc.vector.memset(acc3[:], 0.0)
                    for h in range(2):
                        nc.vector.scalar_tensor_tensor(
                            out=acc3[:, h], in0=Lt[:, h * P:(h + 1) * P],
                            scalar=pself[:, h:h + 1], op0=OP.mult,
                            in1=acc3[:, h], op1=OP.add)
                    x3f = pb.tile([P, P], F32, tag="x3f")
                    x3h = pb.tile([P, 2, P], F32, tag="x3h")
                    nc.vector.tensor_tensor(
                        out=x3h[:], in0=acc3[:],
                        in1=rd[:, :, None].to_broadcast([P, 2, P]), op=OP.mult)
                    nc.vector.tensor_tensor(out=x3f[:], in0=x3h[:, 0],
                                            in1=x3h[:, 1], op=OP.add)
                    oh = pb.tile([P, B], F32, tag="oh")
                    nc.vector.tensor_tensor(
                        out=oh[:], in0=batchf_s[:, t:t + 1].to_broadcast([P, B]),
                        in1=iota_s[:], op=OP.is_equal)
                    nc.tensor.matmul(zp[:], lhsT=x3f[:], rhs=oh[:],
                                     start=(t == 0), stop=(t == T - 1))
                zs = cp.tile([P, B], F32)
                nc.vector.tensor_copy(out=zs[:], in_=zp[:])
                nc.sync.dma_start(out=zT[:], in_=zs[:])
    nc.compile()
    return nc


def _build_L4(meta):
    nc = bacc.Bacc("TRN2", target_bir_lowering=False, debug=False, num_devices=NC)
    zparts = nc.dram_tensor("zparts", [P, NC * B], F32, kind="ExternalInput")
    invcnt = nc.dram_tensor("invcnt", [P, B], F32, kind="ExternalInput")
    gb = nc.dram_tensor("gb", [P, 1], F32, kind="ExternalInput")
    names = ["d1wTa", "d1wTb", "d2wTa", "d2wTb"]
    wts = {n: nc.dram_tensor(n, [P, P], F32, kind="ExternalInput") for n in names}
    d3wT = nc.dram_tensor("d3wT", [P, 40], F32, kind="ExternalInput")
    vecs = {}
    for n in ["d1ba", "d1bb", "bn1ga", "bn1gb", "bn1ba", "bn1bb",
              "d2b", "bn2g", "bn2b"]:
        vecs[n] = nc.dram_tensor(n, [P, 1], F32, kind="ExternalInput")
    for n in ["d3b", "bn3g", "bn3b"]:
        vecs[n] = nc.dram_tensor(n, [40, 1], F32, kind="ExternalInput")
    iota40 = nc.dram_tensor("iota40", [B, 40], F32, kind="ExternalInput")
    ident = nc.dram_tensor("ident", [P, P], F32, kind="ExternalInput")
    zout = nc.dram_tensor("zout", [B, 40], F32, kind="ExternalOutput")
    aout = nc.dram_tensor("aout", [B, 1], I32, kind="ExternalOutput")

    def bn_block(nc, cp, zd, g_ap, b_ap, relu, nparts, eps_s=[None]):
        if eps_s[0] is None:
            e_t = cp.tile([P, 1], F32, tag="epsc", name="epsc")
            nc.vector.memset(e_t[:], EPS)
            eps_s[0] = e_t
        sm = cp.tile([P, 1], F32, tag="sm")
        nc.vector.tensor_reduce(out=sm[:nparts], in_=zd[:nparts],
                                axis=mybir.AxisListType.X, op=OP.add)
        mn = cp.tile([P, 1], F32, tag="mn")
        nc.vector.tensor_scalar(out=mn[:nparts], in0=sm[:nparts],
                                scalar1=1.0 / B, scalar2=None, op0=OP.mult)
        sq = cp.tile([P, B], F32, tag="sq")
        nc.scalar.activation(sq[:nparts], zd[:nparts], AF.Square)
        msq = cp.tile([P, 1], F32, tag="msq")
        nc.vector.tensor_reduce(out=msq[:nparts], in_=sq[:nparts],
                                axis=mybir.AxisListType.X, op=OP.add)
        nc.vector.tensor_scalar(out=msq[:nparts], in0=msq[:nparts],
                                scalar1=1.0 / B, scalar2=None, op0=OP.mult)
        mn2 = cp.tile([P, 1], F32, tag="mn2")
        nc.vector.tensor_tensor(out=mn2[:nparts], in0=mn[:nparts],
                                in1=mn[:nparts], op=OP.mult)
        v = cp.tile([P, 1], F32, tag="v")
        nc.vector.tensor_tensor(out=v[:nparts], in0=msq[:nparts],
                                in1=mn2[:nparts], op=OP.subtract)
        sd = cp.tile([P, 1], F32, tag="sd")
        nc.scalar.activation(sd[:nparts], v[:nparts], AF.Sqrt,
                             bias=eps_s[0][:nparts, :1])
        rs = cp.tile([P, 1], F32, tag="rs")
        nc.vector.reciprocal(rs[:nparts], sd[:nparts])
        s = cp.tile([P, 1], F32, tag="s")
        nc.vector.tensor_tensor(out=s[:nparts], in0=rs[:nparts],
                                in1=g_ap, op=OP.mult)
        ms = cp.tile([P, 1], F32, tag="ms")
        nc.vector.tensor_tensor(out=ms[:nparts], in0=mn[:nparts],
                                in1=s[:nparts], op=OP.mult)
        tsh = cp.tile([P, 1], F32, tag="tsh")
        nc.vector.tensor_tensor(out=tsh[:nparts], in0=b_ap,
                                in1=ms[:nparts], op=OP.subtract)
        zn = cp.tile([P, B], F32, tag="zn" + str(relu))
        nc.vector.tensor_scalar(out=zn[:nparts], in0=zd[:nparts],
                                scalar1=s[:nparts, :1], scalar2=tsh[:nparts, :1],
                                op0=OP.mult, op1=OP.add)
        if relu:
            nc.vector.tensor_scalar(out=zn[:nparts], in0=zn[:nparts],
                                    scalar1=0.0, scalar2=None, op0=OP.max)
        return zn

    with tile.TileContext(nc) as tc:
        with tc.tile_pool(name="c", bufs=1) as cp, \
             tc.tile_pool(name="ps", bufs=1, space="PSUM") as pp:
            zp_s = cp.tile([P, NC * B], F32)
            nc.sync.dma_start(out=zp_s[:], in_=zparts[:])
            ic_s = cp.tile([P, B], F32)
            nc.sync.dma_start(out=ic_s[:], in_=invcnt[:])
            gb_s = cp.tile([P, 1], F32)
            nc.sync.dma_start(out=gb_s[:], in_=gb[:])
            w_s = {}
            for n in names:
                w_s[n] = cp.tile([P, P], F32, tag=n, name=n)
                nc.sync.dma_start(out=w_s[n][:], in_=wts[n][:])
            d3w_s = cp.tile([P, 40], F32)
            nc.sync.dma_start(out=d3w_s[:], in_=d3wT[:])
            v_s = {}
            for n, t_ in vecs.items():
                rows = t_.shape[0]
                v_s[n] = cp.tile([rows, 1], F32, tag="v" + n, name="v" + n)
                nc.sync.dma_start(out=v_s[n][:], in_=t_[:])
            io_s = cp.tile([B, 40], F32)
            nc.sync.dma_start(out=io_s[:], in_=iota40[:])
            id_s = cp.tile([P, P], F32)
            nc.sync.dma_start(out=id_s[:], in_=ident[:])

            zt = cp.tile([P, B], F32)
            nc.vector.tensor_tensor(out=zt[:], in0=zp_s[:, 0:B],
                                    in1=zp_s[:, B:2 * B], op=OP.add)
            for i in range(2, NC):
                nc.vector.tensor_tensor(out=zt[:], in0=zt[:],
                                        in1=zp_s[:, i * B:(i + 1) * B], op=OP.add)
            nc.vector.tensor_tensor(out=zt[:], in0=zt[:], in1=ic_s[:], op=OP.mult)
            nc.vector.tensor_scalar(out=zt[:], in0=zt[:], scalar1=gb_s[:, :1],
                                    scalar2=None, op0=OP.add)

            # d1 + bn1 + relu (two partition-halves of the 256 channels)
            zn1 = []
            for half, (wn, bn_, gn, bbn) in enumerate(
                    [("d1wTa", "d1ba", "bn1ga", "bn1ba"),
                     ("d1wTb", "d1bb", "bn1gb", "bn1bb")]):
                p1 = pp.tile([P, B], F32, space="PSUM", tag="p1" + str(half))
                nc.tensor.matmul(p1[:], lhsT=w_s[wn][:], rhs=zt[:],
                                 start=True, stop=True)
                zd = cp.tile([P, B], F32, tag="zd1" + str(half))
                nc.vector.tensor_scalar(out=zd[:], in0=p1[:],
                                        scalar1=v_s[bn_][:, :1], scalar2=None,
                                        op0=OP.add)
                zn1.append(bn_block(nc, cp, zd, v_s[gn][:, :1], v_s[bbn][:, :1],
                                    True, P))

            # d2 + bn2 + relu
            p2 = pp.tile([P, B], F32, space="PSUM", tag="p2")
            nc.tensor.matmul(p2[:], lhsT=w_s["d2wTa"][:], rhs=zn1[0][:],
                             start=True, stop=False)
            nc.tensor.matmul(p2[:], lhsT=w_s["d2wTb"][:], rhs=zn1[1][:],
                             start=False, stop=True)
            zd2 = cp.tile([P, B], F32)
            nc.vector.tensor_scalar(out=zd2[:], in0=p2[:],
                                    scalar1=v_s["d2b"][:, :1], scalar2=None,
                                    op0=OP.add)
            zn2 = bn_block(nc, cp, zd2, v_s["bn2g"][:, :1], v_s["bn2b"][:, :1],
                           True, P)

            # d3 + bn3
            p3 = pp.tile([40, B], F32, space="PSUM", tag="p3")
            nc.tensor.matmul(p3[:], lhsT=d3w_s[:], rhs=zn2[:],
                             start=True, stop=True)
            zd3 = cp.tile([40, B], F32)
            nc.vector.tensor_scalar(out=zd3[:], in0=p3[:],
                                    scalar1=v_s["d3b"][:, :1], scalar2=None,
                                    op0=OP.add)
            zn3 = bn_block(nc, cp, zd3, v_s["bn3g"][:, :1], v_s["bn3b"][:, :1],
                           False, 40)

            # transpose -> [64, 40], store z, argmax
            ptr = pp.tile([B, 40], F32, space="PSUM", tag="ptr")
            nc.tensor.transpose(out=ptr[:], in_=zn3[:40, :], identity=id_s[:40, :40])
            zf = cp.tile([B, 40], F32)
            nc.vector.tensor_copy(out=zf[:], in_=ptr[:])
            nc.sync.dma_start(out=zout[:], in_=zf[:])
            rmax = cp.tile([B, 1], F32)
            nc.vector.tensor_reduce(out=rmax[:], in_=zf[:],
                                    axis=mybir.AxisListType.X, op=OP.max)
            eqm = cp.tile([B, 40], F32)
            nc.vector.tensor_scalar(out=eqm[:], in0=zf[:],
                                    scalar1=rmax[:, :1], scalar2=None,
                                    op0=OP.is_ge)
            sel = cp.tile([B, 40], F32)
            nc.vector.tensor_scalar(out=sel[:], in0=eqm[:], scalar1=-1e4,
                                    scalar2=1e4, op0=OP.mult, op1=OP.add)
            nc.vector.tensor_tensor(out=sel[:], in0=sel[:], in1=io_s[:], op=OP.add)
            am = cp.tile([B, 1], F32)
            nc.vector.tensor_reduce(out=am[:], in_=sel[:],
                                    axis=mybir.AxisListType.X, op=OP.min)
            ai = cp.tile([B, 1], I32)
            nc.vector.tensor_copy(out=ai[:], in_=am[:])
            nc.sync.dma_start(out=aout[:], in_=ai[:])
    nc.compile()
    return nc


# ------------------------------------------------------------------- driver

def _program(key, builder, meta):
    sig = (key, meta['NPAD'], meta['SC'], tuple(meta['K_sched']))
    if sig not in _programs:
        _programs[sig] = builder(meta)
    return _programs[sig]


def _run(nc, in_maps, tag=None):
    import os
    import time as _time
    t0 = _time.perf_counter()
    res = run_bass_kernel_spmd(nc, in_maps, core_ids=list(range(NC)))
    t1 = _time.perf_counter()
    if os.environ.get("KERNEL_TRACE"):
        LAST.setdefault('times', {})[tag] = int((t1 - t0) * 1e9)
    return res.results


def kernel(pos, edge_index, batch, s1_wl, s1_wr, s1_b, s2_wl, s2_wr, s2_b,
           g_w, g_asrc, g_adst, g_b, d1_w, d1_b, bn1_g, bn1_b,
           d2_w, d2_b, bn2_g, bn2_b, d3_w, d3_b, bn3_g, bn3_b):
    f32 = np.float32
    pos = np.asarray(pos, f32)
    meta, percore = _prep(pos, edge_index, batch)
    NPAD, T, NT = meta['NPAD'], meta['T'], meta['NT']
    new = meta['new_of_old']

    ident = np.eye(P, dtype=f32)
    iota64 = np.tile(np.arange(B, dtype=f32), (P, 1))
    iota40 = np.tile(np.arange(40, dtype=f32), (B, 1))

    # pos4 table [4, NT] in permuted order (pad cols zero)
    pos4 = np.zeros((4, NT), f32)
    pos4[:3, new] = pos.T
    pos4[3, new] = 1.0

    w1l = np.concatenate([np.asarray(s1_wl, f32).T, np.zeros((1, P), f32)], 0)
    w1r = np.concatenate([np.asarray(s1_wr, f32).T,
                          np.asarray(s1_b, f32)[None, :]], 0)

    L1 = _program("L1", _build_L1, meta)
    in_maps = []
    for c in range(NC):
        pc = percore[c]
        p4rot = _rotate(pos4, c, NPAD)
        in_maps.append(dict(
            pos4r=np.ascontiguousarray(p4rot.T), pos4o=p4rot[:, :NPAD].copy(),
            w1l=w1l, w1r=w1r, idx=pc['idx'],
            invd=pc['invd'], realm=pc['realm'], ident=ident))
    r1 = _run(L1, in_maps, "L1")
    x1T_full = np.concatenate([r1[c]["x1T"] for c in range(NC)], axis=1)
    x1R_full = np.concatenate([r1[c]["x1R"] for c in range(NC)], axis=0)
    LAST['x1T'] = x1T_full

    L2 = _program("L2", _build_L2, meta)
    in_maps = []
    for c in range(NC):
        pc = percore[c]
        in_maps.append(dict(
            x1r=_rotate(x1R_full, c, NPAD, axis=0),
            x1To=x1T_full[:, c * NPAD:(c + 1) * NPAD].copy(),
            w2l=np.asarray(s2_wl, f32).T.copy(),
            w2r=np.asarray(s2_wr, f32).T.copy(),
            s2b=np.asarray(s2_b, f32)[None, :], idx=pc['idx'],
            invd=pc['invd'], realm=pc['realm'], ident=ident))
    r2 = _run(L2, in_maps, "L2")
    x2T_full = np.concatenate([r2[c]["x2T"] for c in range(NC)], axis=1)
    LAST['x2T'] = x2T_full

    # W_ext [256, 260]
    g_w = np.asarray(g_w, f32)
    H, C = 2, 128
    w_asrc = np.zeros((256, H), f32)
    w_adst = np.zeros((256, H), f32)
    ga, gd = np.asarray(g_asrc, f32), np.asarray(g_adst, f32)
    for h in range(H):
        w_asrc[:, h] = g_w[h * C:(h + 1) * C, :].T @ ga[h]
        w_adst[:, h] = g_w[h * C:(h + 1) * C, :].T @ gd[h]
    W_ext = np.concatenate([g_w.T, w_asrc, w_adst], axis=1)  # [256, 260]

    L3 = _program("L3", _build_L3, meta)
    in_maps = []
    for c in range(NC):
        pc = percore[c]
        in_maps.append(dict(
            x1T=_rotate(x1T_full, c, NPAD), x2T=_rotate(x2T_full, c, NPAD),
            w3a=W_ext[:128].copy(), w3b=W_ext[128:].copy(),
            idx=pc['idx'], mask=pc['mask'], invdsl=pc['invdsl'],
            batchf=pc['batchf'], iota64=iota64))
    r3 = _run(L3, in_maps, "L3")
    zparts = np.concatenate([r3[c]["zT"] for c in range(NC)], axis=1)
    LAST['zparts'] = zparts

    L4 = _program("L4", _build_L4, meta)
    d1_w = np.asarray(d1_w, f32)
    d2_w = np.asarray(d2_w, f32)
    d3_w = np.asarray(d3_w, f32)
    col = lambda a: np.asarray(a, f32).reshape(-1, 1)
    in_map = dict(
        zparts=zparts, invcnt=np.tile(meta['inv_cnt'][None, :], (P, 1)),
        gb=col(g_b),
        d1wTa=d1_w.T[:, :128].copy(), d1wTb=d1_w.T[:, 128:].copy(),
        d2wTa=d2_w[:, :128].T.copy(), d2wTb=d2_w[:, 128:].T.copy(),
        d3wT=d3_w.T.copy(),
        d1ba=col(d1_b[:128]), d1bb=col(d1_b[128:]),
        bn1ga=col(bn1_g[:128]), bn1gb=col(bn1_g[128:]),
        bn1ba=col(bn1_b[:128]), bn1bb=col(bn1_b[128:]),
        d2b=col(d2_b), bn2g=col(bn2_g), bn2b=col(bn2_b),
        d3b=col(d3_b), bn3g=col(bn3_g), bn3b=col(bn3_b),
        iota40=iota40, ident=ident)
    r4 = _run(L4, [in_map] * NC, "L4")
    z = r4[0]["zout"]
    amax = r4[0]["aout"].reshape(B).astype(np.int32)
    if os.environ.get("KERNEL_PREDICT"):
        try:
            from concourse.timeline_sim import TimelineSim
            tot = 0.0
            for key in ["L1", "L2", "L3", "L4"]:
                sig = (key, meta['NPAD'], meta['SC'], tuple(meta['K_sched']))
                tot += TimelineSim(_programs[sig], trace=False).simulate()
            LAST['predicted_ns'] = int(tot)
        except Exception:
            pass
    return z, amax


# revision 16
# speedup vs baseline: 25508.9896x; 25508.9896x over previous
"""Trainium2 Bass kernel for nn_GNN_82171314307289 (2x SAGEConv + GAT + pool + MLP).

8-core graph-parallel. Key ideas:
  - Nodes partitioned by owner core (8 graphs/core via sorted `batch`),
    degree-sorted within core, padded to NPAD/core. All index prep on host.
  - Aggregations via indirect_dma_start in its [128,1]-offset form (the only
    indirect mode this stack executes correctly): one 128-row gather per
    (node-tile, slot); K-slot layout over degree-sorted tiles minimizes the
    slot count (SC ~ E/(8*128) per core), slot sums reduced on DVE.
  - SAGE layers gather SOURCE rows directly (mean commutes with the linear
    layer): L1 gathers 16B pos rows, L2 gathers x1 rows, the per-tile
    aggregate is transposed once on the PE and folded into the same PSUM
    matmul as the root/bias terms -- no materialized y-tables.
  - GAT gathers rows of an xw table [N, 260] = x3 @ [g_w.T | w_asrc | w_adst]
    (attention coefficients folded in as extra weight columns, host-folded
    from g_w/g_asrc/g_adst); self-loop uses the local row load; masked
    softmax + weighted message sum on DVE/ACT per tile; 0.5 head-mean and
    g_b folded downstream; mean-pool via one-hot matmul into PSUM.
  - Per-core tables/inputs are laid out in ROTATED shard order (slab j =
    core (c+j)%8) so the SPMD program is core-independent (own = slab 0).
  - 4 launches (x1 | x2 | GAT+pool | decoder+argmax); the host relays
    activation shards between launches (pure data movement, all flops on
    device). BatchNorm/argmax run per-partition-scalar tricks in transposed
    [channel, graph] layout on one core's worth of data, replicated.
"""
import math
import os
import numpy as np

ABL = set(os.environ.get("ABL", "").split(","))

import concourse.bass as bass
import concourse.bacc as bacc
import concourse.tile as tile
import concourse.mybir as mybir
from concourse.bass_utils import run_bass_kernel_spmd

NC = 8
P = 128
B = 64
EPS = 1e-5
F32 = mybir.dt.float32
I32 = mybir.dt.int32
AF = mybir.ActivationFunctionType
OP = mybir.AluOpType
XW_W = 260          # xw table row: 256 xw | 2 a_src | 2 a_dst

_programs = {}
LAST = {}


# ------------------------------------------------------------------ host prep

def _prep(pos, edge_index, batch):
    N = pos.shape[0]
    src = np.asarray(edge_index[0], dtype=np.int64)
    dst = np.asarray(edge_index[1], dtype=np.int64)
    batch = np.asarray(batch, dtype=np.int64)

    owner = batch // (B // NC)
    deg = np.bincount(dst, minlength=N)
    counts = np.bincount(owner, minlength=NC)
    NPAD = int(math.ceil((counts.max() + 16) / 128.0) * 128)
    T = NPAD // P
    NT = NC * NPAD
    ZR = NPAD - 1                     # guaranteed pad node -> all-zero table row

    # permutation: per core, degree-descending
    new_of_old = np.empty(N, dtype=np.int64)
    for c in range(NC):
        nodes = np.where(owner == c)[0]
        order = nodes[np.argsort(-deg[nodes], kind="stable")]
        new_of_old[order] = c * NPAD + np.arange(order.shape[0])

    deg_n = np.zeros(NC * NPAD, dtype=np.int64)
    deg_n[new_of_old] = deg
    batch_n = np.full(NC * NPAD, B, dtype=np.int64)
    batch_n[new_of_old] = batch
    real_n = np.zeros(NC * NPAD, dtype=np.float32)
    real_n[new_of_old] = 1.0

    src_n = new_of_old[src]
    dst_n = new_of_old[dst]
    c_dst = dst_n // NPAD
    dl = dst_n % NPAD

    # K schedule (shared across cores)
    K_tile = deg_n.reshape(NC, T, P).max(axis=2)          # [NC, T]
    K_sched = K_tile.max(axis=0).astype(np.int64)         # [T]
    SC = int(K_sched.sum())
    coff = np.zeros(T + 1, dtype=np.int64)
    coff[1:] = np.cumsum(K_sched)

    # slot fill: edges sorted by (core, dst, id) -> slot index per dst
    eorder = np.lexsort((np.arange(src.shape[0]), dst_n))
    ds = dst_n[eorder]
    ss = src_n[eorder]
    first = np.ones(ds.shape[0], dtype=bool)
    first[1:] = ds[1:] != ds[:-1]
    gi = np.arange(ds.shape[0])
    run0 = np.maximum.accumulate(np.where(first, gi, 0))
    slot = gi - run0

    # per-core idx + mask arrays [128, SC]
    idx_all, mask_all = [], []
    s_core = ss // NPAD
    s_loc = ss % NPAD
    d_core = ds // NPAD
    d_loc = ds % NPAD
    d_t = d_loc // P
    d_p = d_loc % P
    for c in range(NC):
        idx = np.full((P, SC), ZR, dtype=np.int64)        # pad -> zero row slab0
        msk = np.zeros((P, SC), dtype=np.float32)
        m = d_core == c
        j = (s_core[m] - c) % NC
        rot = j * NPAD + s_loc[m]
        col = coff[d_t[m]] + slot[m]
        idx[d_p[m], col] = rot
        msk[d_p[m], col] = 1.0
        idx_all.append(idx.astype(np.int32))
        mask_all.append(msk)

    def tilewise(arr, dtype=np.float32):
        return arr.reshape(NC, T, P).transpose(0, 2, 1).astype(dtype)  # [NC,128,T]

    invd = tilewise(1.0 / np.maximum(deg_n.astype(np.float64), 1.0))
    invdsl = tilewise(0.5 / (deg_n.astype(np.float64) + 1.0))
    batchf = tilewise(batch_n.astype(np.float64))
    realm = tilewise(real_n.astype(np.float64))

    cnt_graph = np.bincount(batch, minlength=B).astype(np.float64)
    inv_cnt = (1.0 / np.maximum(cnt_graph, 1.0)).astype(np.float32)

    meta = dict(N=N, NPAD=NPAD, T=T, NT=NT, SC=SC,
                K_sched=K_sched.tolist(), coff=coff.tolist(),
                new_of_old=new_of_old, inv_cnt=inv_cnt)
    percore = [dict(idx=idx_all[c], mask=mask_all[c], invd=invd[c],
                    invdsl=invdsl[c], batchf=batchf[c], realm=realm[c])
               for c in range(NC)]
    return meta, percore


def _rotate(full, c, NPAD, axis=1):
    """Rotate shards along `axis` so own (core c) is slab 0."""
    k = c * NPAD
    if axis == 0:
        return np.concatenate([full[k:], full[:k]], axis=0)
    return np.concatenate([full[:, k:], full[:, :k]], axis=1)


# ------------------------------------------------------------- bass programs

def _common_build(nc, meta, idx_name="idx", with_mask=False):
    pass


def _build_L1(meta):
    NPAD, T, NT, SC = meta['NPAD'], meta['T'], meta['NT'], meta['SC']
    K_sched, coff = meta['K_sched'], meta['coff']
    nc = bacc.Bacc("TRN2", target_bir_lowering=False, debug=False, num_devices=NC)
    pos4r = nc.dram_tensor("pos4r", [NT, 4], F32, kind="ExternalInput")
    pos4o = nc.dram_tensor("pos4o", [4, NPAD], F32, kind="ExternalInput")
    w1l = nc.dram_tensor("w1l", [4, P], F32, kind="ExternalInput")
    w1r = nc.dram_tensor("w1r", [4, P], F32, kind="ExternalInput")
    idx = nc.dram_tensor("idx", [P, SC], I32, kind="ExternalInput")
    invd = nc.dram_tensor("invd", [P, T], F32, kind="ExternalInput")
    realm = nc.dram_tensor("realm", [P, T], F32, kind="ExternalInput")
    ident = nc.dram_tensor("ident", [P, P], F32, kind="ExternalInput")
    x1T = nc.dram_tensor("x1T", [P, NPAD], F32, kind="ExternalOutput")
    x1R = nc.dram_tensor("x1R", [NPAD, P], F32, kind="ExternalOutput")

    with tile.TileContext(nc) as tc:
        with tc.tile_pool(name="const", bufs=1) as cp:
            w1l_s = cp.tile([4, P], F32)
            w1r_s = cp.tile([4, P], F32)
            p4o_s = cp.tile([4, NPAD], F32)
            id_s = cp.tile([P, P], F32)
            invd_s = cp.tile([P, T], F32)
            realm_s = cp.tile([P, T], F32)
            idx_s = cp.tile([P, SC], I32)
            nc.sync.dma_start(out=w1l_s[:], in_=w1l[:])
            nc.sync.dma_start(out=w1r_s[:], in_=w1r[:])
            nc.sync.dma_start(out=p4o_s[:], in_=pos4o[:])
            nc.sync.dma_start(out=id_s[:], in_=ident[:])
            nc.sync.dma_start(out=invd_s[:], in_=invd[:])
            nc.sync.dma_start(out=realm_s[:], in_=realm[:])
            nc.sync.dma_start(out=idx_s[:], in_=idx[:])

            with tc.tile_pool(name="pb", bufs=4) as pb, \
                 tc.tile_pool(name="pbp", bufs=3, space="PSUM") as pbp, \
                 tc.tile_pool(name="pbt", bufs=2, space="PSUM") as pbt, \
                 tc.tile_pool(name="pba", bufs=2, space="PSUM") as pba:
                for t in range(T):
                    K = K_sched[t]
                    rp = pbp.tile([P, P], F32, space="PSUM", tag="r")
                    if K > 0:
                        G = pb.tile([P, max(K, 2), 4], F32, tag="G")
                        for k in range(K):
                            nc.gpsimd.indirect_dma_start(
                                out=G[:, k, :], out_offset=None, in_=pos4r[:],
                                in_offset=bass.IndirectOffsetOnAxis(
                                    ap=idx_s[:, coff[t] + k:coff[t] + k + 1],
                                    axis=0))
                        acc = pb.tile([P, 4], F32, tag="acc")
                        if K == 1:
                            nc.vector.tensor_copy(out=acc[:], in_=G[:, 0, :])
                        else:
                            nc.vector.tensor_tensor(out=acc[:], in0=G[:, 0, :],
                                                    in1=G[:, 1, :], op=OP.add)
                            for k in range(2, K):
                                nc.vector.tensor_tensor(out=acc[:], in0=acc[:],
                                                        in1=G[:, k, :], op=OP.add)
                        nc.vector.tensor_scalar(
                            out=acc[:], in0=acc[:], scalar1=invd_s[:, t:t + 1],
                            scalar2=None, op0=OP.mult)
                        ta = pba.tile([4, P], F32, space="PSUM", tag="ta")
                        nc.tensor.transpose(out=ta[:], in_=acc[:], identity=id_s[:])
                        aggT = pb.tile([4, P], F32, tag="aggT")
                        nc.scalar.copy(out=aggT[:], in_=ta[:])
                        nc.tensor.matmul(rp[:], lhsT=aggT[:], rhs=w1l_s[:],
                                         start=True, stop=False)
                        nc.tensor.matmul(rp[:], lhsT=p4o_s[:, t * P:(t + 1) * P],
                                         rhs=w1r_s[:], start=False, stop=True)
                    else:
                        nc.tensor.matmul(rp[:], lhsT=p4o_s[:, t * P:(t + 1) * P],
                                         rhs=w1r_s[:], start=True, stop=True)
                    x1t = pb.tile([P, P], F32, tag="x1t")
                    nc.vector.tensor_scalar(
                        out=x1t[:], in0=rp[:], scalar1=0.0,
                        scalar2=realm_s[:, t:t + 1], op0=OP.max, op1=OP.mult)
                    nc.sync.dma_start(out=x1R[t * P:(t + 1) * P, :], in_=x1t[:])
                    tp = pbt.tile([P, P], F32, space="PSUM", tag="tr")
                    nc.tensor.transpose(out=tp[:], in_=x1t[:], identity=id_s[:])
                    xo = pb.tile([P, P], F32, tag="xo")
                    nc.scalar.copy(out=xo[:], in_=tp[:])
                    nc.sync.dma_start(out=x1T[:, t * P:(t + 1) * P], in_=xo[:])
    nc.compile()
    return nc


def _build_L2(meta):
    NPAD, T, NT, SC = meta['NPAD'], meta['T'], meta['NT'], meta['SC']
    K_sched, coff = meta['K_sched'], meta['coff']
    nc = bacc.Bacc("TRN2", target_bir_lowering=False, debug=False, num_devices=NC)
    x1r = nc.dram_tensor("x1r", [NT, P], F32, kind="ExternalInput")
    x1To = nc.dram_tensor("x1To", [P, NPAD], F32, kind="ExternalInput")
    w2l = nc.dram_tensor("w2l", [P, P], F32, kind="ExternalInput")
    w2r = nc.dram_tensor("w2r", [P, P], F32, kind="ExternalInput")
    s2b = nc.dram_tensor("s2b", [1, P], F32, kind="ExternalInput")
    idx = nc.dram_tensor("idx", [P, SC], I32, kind="ExternalInput")
    invd = nc.dram_tensor("invd", [P, T], F32, kind="ExternalInput")
    realm = nc.dram_tensor("realm", [P, T], F32, kind="ExternalInput")
    ident = nc.dram_tensor("ident", [P, P], F32, kind="ExternalInput")
    x2T = nc.dram_tensor("x2T", [P, NPAD], F32, kind="ExternalOutput")

    with tile.TileContext(nc) as tc:
        with tc.tile_pool(name="const", bufs=1) as cp:
            w2l_s = cp.tile([P, P], F32)
            w2r_s = cp.tile([P, P], F32)
            s2b_s = cp.tile([1, P], F32)
            ones_s = cp.tile([1, P], F32)
            id_s = cp.tile([P, P], F32)
            invd_s = cp.tile([P, T], F32)
            realm_s = cp.tile([P, T], F32)
            idx_s = cp.tile([P, SC], I32)
            nc.sync.dma_start(out=w2l_s[:], in_=w2l[:])
            nc.sync.dma_start(out=w2r_s[:], in_=w2r[:])
            nc.sync.dma_start(out=s2b_s[:], in_=s2b[:])
            nc.vector.memset(ones_s[:], 1.0)
            nc.sync.dma_start(out=id_s[:], in_=ident[:])
            nc.sync.dma_start(out=invd_s[:], in_=invd[:])
            nc.sync.dma_start(out=realm_s[:], in_=realm[:])
            nc.sync.dma_start(out=idx_s[:], in_=idx[:])

            with tc.tile_pool(name="pb", bufs=4) as pb, \
                 tc.tile_pool(name="pbp", bufs=3, space="PSUM") as pbp, \
                 tc.tile_pool(name="pbt", bufs=2, space="PSUM") as pbt, \
                 tc.tile_pool(name="pba", bufs=2, space="PSUM") as pba:
                for t in range(T):
                    K = K_sched[t]
                    xl = pb.tile([P, P], F32, tag="xl")
                    nc.sync.dma_start(out=xl[:], in_=x1To[:, t * P:(t + 1) * P])
                    rp = pbp.tile([P, P], F32, space="PSUM", tag="r")
                    nc.tensor.matmul(rp[:], lhsT=xl[:], rhs=w2r_s[:],
                                     start=True, stop=False)
                    nc.tensor.matmul(rp[:], lhsT=ones_s[:], rhs=s2b_s[:],
                                     start=False, stop=(K == 0))
                    if K > 0:
                        G = pb.tile([P, max(K, 2), P], F32, tag="G")
                        for k in range(K):
                            nc.gpsimd.indirect_dma_start(
                                out=G[:, k, :], out_offset=None, in_=x1r[:],
                                in_offset=bass.IndirectOffsetOnAxis(
                                    ap=idx_s[:, coff[t] + k:coff[t] + k + 1],
                                    axis=0))
                        acc = pb.tile([P, P], F32, tag="acc")
                        if K == 1:
                            nc.vector.tensor_copy(out=acc[:], in_=G[:, 0, :])
                        else:
                            nc.vector.tensor_tensor(out=acc[:], in0=G[:, 0, :],
                                                    in1=G[:, 1, :], op=OP.add)
                            for k in range(2, K):
                                nc.vector.tensor_tensor(out=acc[:], in0=acc[:],
                                                        in1=G[:, k, :], op=OP.add)
                        nc.vector.tensor_scalar(
                            out=acc[:], in0=acc[:], scalar1=invd_s[:, t:t + 1],
                            scalar2=None, op0=OP.mult)
                        ta = pba.tile([P, P], F32, space="PSUM", tag="ta")
                        nc.tensor.transpose(out=ta[:], in_=acc[:], identity=id_s[:])
                        aggT = pb.tile([P, P], F32, tag="aggT")
                        nc.scalar.copy(out=aggT[:], in_=ta[:])
                        nc.tensor.matmul(rp[:], lhsT=aggT[:], rhs=w2l_s[:],
                                         start=False, stop=True)
                    x2t = pb.tile([P, P], F32, tag="x2t")
                    nc.vector.tensor_scalar(
                        out=x2t[:], in0=rp[:], scalar1=0.0,
                        scalar2=realm_s[:, t:t + 1], op0=OP.max, op1=OP.mult)
                    tp = pbt.tile([P, P], F32, space="PSUM", tag="tr")
                    nc.tensor.transpose(out=tp[:], in_=x2t[:], identity=id_s[:])
                    xo = pb.tile([P, P], F32, tag="xo")
                    nc.scalar.copy(out=xo[:], in_=tp[:])
                    nc.sync.dma_start(out=x2T[:, t * P:(t + 1) * P], in_=xo[:])
    nc.compile()
    return nc


def _build_L3(meta):
    NPAD, T, NT, SC = meta['NPAD'], meta['T'], meta['NT'], meta['SC']
    K_sched, coff = meta['K_sched'], meta['coff']
    W = XW_W
    nc = bacc.Bacc("TRN2", target_bir_lowering=False, debug=False, num_devices=NC)
    x1T = nc.dram_tensor("x1T", [P, NT], F32, kind="ExternalInput")
    x2T = nc.dram_tensor("x2T", [P, NT], F32, kind="ExternalInput")
    w3a = nc.dram_tensor("w3a", [P, W], F32, kind="ExternalInput")
    w3b = nc.dram_tensor("w3b", [P, W], F32, kind="ExternalInput")
    idx = nc.dram_tensor("idx", [P, SC], I32, kind="ExternalInput")
    mask = nc.dram_tensor("mask", [P, SC], F32, kind="ExternalInput")
    invdsl = nc.dram_tensor("invdsl", [P, T], F32, kind="ExternalInput")
    batchf = nc.dram_tensor("batchf", [P, T], F32, kind="ExternalInput")
    iota64 = nc.dram_tensor("iota64", [P, B], F32, kind="ExternalInput")
    zT = nc.dram_tensor("zT", [P, B], F32, kind="ExternalOutput")
    xw = nc.dram_tensor("xwtab", [NT, W], F32, kind="Internal")

    with tile.TileContext(nc) as tc:
        with tc.tile_pool(name="const", bufs=1) as cp:
            w3a_s = cp.tile([P, W], F32)
            w3b_s = cp.tile([P, W], F32)
            invdsl_s = cp.tile([P, T], F32)
            batchf_s = cp.tile([P, T], F32)
            iota_s = cp.tile([P, B], F32)
            idx_s = cp.tile([P, SC], I32)
            mask_s = cp.tile([P, SC], F32)
            nc.sync.dma_start(out=w3a_s[:], in_=w3a[:])
            nc.sync.dma_start(out=w3b_s[:], in_=w3b[:])
            nc.sync.dma_start(out=invdsl_s[:], in_=invdsl[:])
            nc.sync.dma_start(out=batchf_s[:], in_=batchf[:])
            nc.sync.dma_start(out=iota_s[:], in_=iota64[:])
            nc.sync.dma_start(out=idx_s[:], in_=idx[:])
            nc.sync.dma_start(out=mask_s[:], in_=mask[:])

            # phase A: xw table
            with tc.tile_pool(name="pa", bufs=3) as pa, \
                 tc.tile_pool(name="pap", bufs=4, space="PSUM") as pap:
                for g in range(NT // 512):
                    xs1 = pa.tile([P, 512], F32, tag="xs1")
                    xs2 = pa.tile([P, 512], F32, tag="xs2")
                    nc.sync.dma_start(out=xs1[:], in_=x1T[:, g * 512:(g + 1) * 512])
                    nc.sync.dma_start(out=xs2[:], in_=x2T[:, g * 512:(g + 1) * 512])
                    for u in range(4):
                        wp = pap.tile([P, W], F32, space="PSUM")
                        nc.tensor.matmul(wp[:], lhsT=xs1[:, u * P:(u + 1) * P],
                                         rhs=w3a_s[:], start=True, stop=False)
                        nc.tensor.matmul(wp[:], lhsT=xs2[:, u * P:(u + 1) * P],
                                         rhs=w3b_s[:], start=False, stop=True)
                        ws = pa.tile([P, W], F32, tag="ws")
                        nc.vector.tensor_copy(out=ws[:], in_=wp[:])
                        nc.sync.dma_start(
                            out=xw[g * 512 + u * P:g * 512 + (u + 1) * P, :],
                            in_=ws[:])

            # phase B: GAT per tile + pooling
            with tc.tile_pool(name="pb", bufs=3) as pb, \
                 tc.tile_pool(name="pz", bufs=1, space="PSUM") as pz:
                zp = pz.tile([P, B], F32, space="PSUM")
                for t in range(T):
                    K = K_sched[t]
                    Lt = pb.tile([P, W], F32, tag="Lt")
                    nc.sync.dma_start(out=Lt[:], in_=xw[t * P:(t + 1) * P, :])
                    # self attention logit
                    es = pb.tile([P, 2], F32, tag="es")
                    nc.vector.tensor_tensor(out=es[:], in0=Lt[:, 256:258],
                                            in1=Lt[:, 258:260], op=OP.add)
                    # leaky_relu(x,0.2) = max(0.2x, x); HW Lrelu ignores alpha
                    nc.vector.scalar_tensor_tensor(
                        out=es[:], in0=es[:], scalar=0.2, op0=OP.mult,
                        in1=es[:], op1=OP.max)
                    acc3 = pb.tile([P, 2, P], F32, tag="acc3")
                    if K > 0:
                        G = pb.tile([P, max(K, 2), W], F32, tag="G")
                        for k in range(K):
                            if "nogather" in ABL:
                                break
                            nc.gpsimd.indirect_dma_start(
                                out=G[:, k, :], out_offset=None, in_=xw[:],
                                in_offset=bass.IndirectOffsetOnAxis(
                                    ap=idx_s[:, coff[t] + k:coff[t] + k + 1],
                                    axis=0))
                        et = pb.tile([P, 2, max(K, 2)], F32, tag="et")
                        nc.vector.tensor_tensor(
                            out=et[:, :, :K],
                            in0=G[:, :K, 256:258].rearrange("p k h -> p h k"),
                            in1=Lt[:, 258:260][:, :, None].to_broadcast([P, 2, K]),
                            op=OP.add)
                        nc.vector.scalar_tensor_tensor(
                            out=et[:, :, :K], in0=et[:, :, :K], scalar=0.2,
                            op0=OP.mult, in1=et[:, :, :K], op1=OP.max)
                        mx = pb.tile([P, 2], F32, tag="mx")
                        nc.vector.tensor_reduce(out=mx[:], in_=et[:, :, :K],
                                                axis=mybir.AxisListType.X,
                                                op=OP.max)
                        nc.vector.tensor_tensor(out=mx[:], in0=mx[:], in1=es[:],
                                                op=OP.max)
                        ngm = pb.tile([P, 2], F32, tag="ngm")
                        nc.vector.tensor_scalar(out=ngm[:], in0=mx[:],
                                                scalar1=-1.0, scalar2=None,
                                                op0=OP.mult)
                        pt = pb.tile([P, 2, max(K, 2)], F32, tag="pt")
                        for h in range(2):
                            nc.scalar.activation(pt[:, h, :K], et[:, h, :K],
                                                 AF.Exp, bias=ngm[:, h:h + 1])
                        nc.vector.tensor_tensor(
                            out=pt[:, :, :K], in0=pt[:, :, :K],
                            in1=mask_s[:, coff[t]:coff[t] + K][:, None, :]
                            .to_broadcast([P, 2, K]),
                            op=OP.mult)
                        # self prob
                        pself = pb.tile([P, 2], F32, tag="pself")
                        nc.vector.tensor_tensor(out=pself[:], in0=es[:],
                                                in1=mx[:], op=OP.subtract)
                        nc.scalar.activation(pself[:], pself[:], AF.Exp)
                        dn = pb.tile([P, 2], F32, tag="dn")
                        nc.vector.tensor_reduce(out=dn[:], in_=pt[:, :, :K],
                                                axis=mybir.AxisListType.X,
                                                op=OP.add)
                        nc.vector.tensor_tensor(out=dn[:], in0=dn[:],
                                                in1=pself[:], op=OP.add)
                        rd = pb.tile([P, 2], F32, tag="rd")
                        nc.vector.reciprocal(rd[:], dn[:])
                        nc.vector.tensor_scalar(out=rd[:], in0=rd[:],
                                                scalar1=invdsl_s[:, t:t + 1],
                                                scalar2=None, op0=OP.mult)
                        # weighted messages
                        tm = pb.tile([P, max(K, 2), 2, P], F32, tag="tm")
                        nc.vector.tensor_tensor(
                            out=tm[:, :K], in0=G[:, :K, 0:256]
                            .rearrange("p k (h c) -> p k h c", h=2),
                            in1=pt[:].rearrange("p h k -> p k h")[:, :K, :, None]
                            .to_broadcast([P, K, 2, P]),
                            op=OP.mult)
                        if K == 1:
                            nc.vector.tensor_copy(out=acc3[:], in_=tm[:, 0])
                        else:
                            nc.vector.tensor_tensor(out=acc3[:], in0=tm[:, 0],
                                                    in1=tm[:, 1], op=OP.add)
                            for k in range(2, K):
                                nc.vector.tensor_tensor(out=acc3[:], in0=acc3[:],
                                                        in1=tm[:, k], op=OP.add)
                    else:
                        pself = pb.tile([P, 2], F32, tag="pself")
                        nc.vector.memset(pself[:], 1.0)
                        rd = pb.tile([P, 2], F32, tag="rd")
                        nc.vector.tensor_scalar(out=rd[:], in0=pself[:],
                                                scalar1=invdsl_s[:, t:t + 1],
                                                scalar2=None, op0=OP.mult)
                        nc.vector.memset(acc3[:], 0.0)
                    for h in range(2):
                        nc.vector.scalar_tensor_tensor(
                            out=acc3[:, h], in0=Lt[:, h * P:(h + 1) * P],
                            scalar=pself[:, h:h + 1], op0=OP.mult,
                            in1=acc3[:, h], op1=OP.add)
                    x3f = pb.tile([P, P], F32, tag="x3f")
                    x3h = pb.tile([P, 2, P], F32, tag="x3h")
                    nc.vector.tensor_tensor(
                        out=x3h[:], in0=acc3[:],
                        in1=rd[:, :, None].to_broadcast([P, 2, P]), op=OP.mult)
                    nc.vector.tensor_tensor(out=x3f[:], in0=x3h[:, 0],
                                            in1=x3h[:, 1], op=OP.add)
                    oh = pb.tile([P, B], F32, tag="oh")
                    nc.vector.tensor_tensor(
                        out=oh[:], in0=batchf_s[:, t:t + 1].to_broadcast([P, B]),
                        in1=iota_s[:], op=OP.is_equal)
                    nc.tensor.matmul(zp[:], lhsT=x3f[:], rhs=oh[:],
                                     start=(t == 0), stop=(t == T - 1))
                zs = cp.tile([P, B], F32)
                nc.vector.tensor_copy(out=zs[:], in_=zp[:])
                nc.sync.dma_start(out=zT[:], in_=zs[:])
    nc.compile()
    return nc


def _build_L4(meta):
    nc = bacc.Bacc("TRN2", target_bir_lowering=False, debug=False, num_devices=NC)
    zparts = nc.dram_tensor("zparts", [P, NC * B], F32, kind="ExternalInput")
    invcnt = nc.dram_tensor("invcnt", [P, B], F32, kind="ExternalInput")
    gb = nc.dram_tensor("gb", [P, 1], F32, kind="ExternalInput")
    names = ["d1wTa", "d1wTb", "d2wTa", "d2wTb"]
    wts = {n: nc.dram_tensor(n, [P, P], F32, kind="ExternalInput") for n in names}
    d3wT = nc.dram_tensor("d3wT", [P, 40], F32, kind="ExternalInput")
    vecs = {}
    for n in ["d1ba", "d1bb", "bn1ga", "bn1gb", "bn1ba", "bn1bb",
              "d2b", "bn2g", "bn2b"]:
        vecs[n] = nc.dram_tensor(n, [P, 1], F32, kind="ExternalInput")
    for n in ["d3b", "bn3g", "bn3b"]:
        vecs[n] = nc.dram_tensor(n, [40, 1], F32, kind="ExternalInput")
    iota40 = nc.dram_tensor("iota40", [B, 40], F32, kind="ExternalInput")
    ident = nc.dram_tensor("ident", [P, P], F32, kind="ExternalInput")
    zout = nc.dram_tensor("zout", [B, 40], F32, kind="ExternalOutput")
    aout = nc.dram_tensor("aout", [B, 1], I32, kind="ExternalOutput")

    def bn_block(nc, cp, zd, g_ap, b_ap, relu, nparts, eps_s=[None]):
        if eps_s[0] is None:
            e_t = cp.tile([P, 1], F32, tag="epsc", name="epsc")
            nc.vector.memset(e_t[:], EPS)
            eps_s[0] = e_t
        sm = cp.tile([P, 1], F32, tag="sm")
        nc.vector.tensor_reduce(out=sm[:nparts], in_=zd[:nparts],
                                axis=mybir.AxisListType.X, op=OP.add)
        mn = cp.tile([P, 1], F32, tag="mn")
        nc.vector.tensor_scalar(out=mn[:nparts], in0=sm[:nparts],
                                scalar1=1.0 / B, scalar2=None, op0=OP.mult)
        sq = cp.tile([P, B], F32, tag="sq")
        nc.scalar.activation(sq[:nparts], zd[:nparts], AF.Square)
        msq = cp.tile([P, 1], F32, tag="msq")
        nc.vector.tensor_reduce(out=msq[:nparts], in_=sq[:nparts],
                                axis=mybir.AxisListType.X, op=OP.add)
        nc.vector.tensor_scalar(out=msq[:nparts], in0=msq[:nparts],
                                scalar1=1.0 / B, scalar2=None, op0=OP.mult)
        mn2 = cp.tile([P, 1], F32, tag="mn2")
        nc.vector.tensor_tensor(out=mn2[:nparts], in0=mn[:nparts],
                                in1=mn[:nparts], op=OP.mult)
        v = cp.tile([P, 1], F32, tag="v")
        nc.vector.tensor_tensor(out=v[:nparts], in0=msq[:nparts],
                                in1=mn2[:nparts], op=OP.subtract)
        sd = cp.tile([P, 1], F32, tag="sd")
        nc.scalar.activation(sd[:nparts], v[:nparts], AF.Sqrt,
                             bias=eps_s[0][:nparts, :1])
        rs = cp.tile([P, 1], F32, tag="rs")
        nc.vector.reciprocal(rs[:nparts], sd[:nparts])
        s = cp.tile([P, 1], F32, tag="s")
        nc.vector.tensor_tensor(out=s[:nparts], in0=rs[:nparts],
                                in1=g_ap, op=OP.mult)
        ms = cp.tile([P, 1], F32, tag="ms")
        nc.vector.tensor_tensor(out=ms[:nparts], in0=mn[:nparts],
                                in1=s[:nparts], op=OP.mult)
        tsh = cp.tile([P, 1], F32, tag="tsh")
        nc.vector.tensor_tensor(out=tsh[:nparts], in0=b_ap,
                                in1=ms[:nparts], op=OP.subtract)
        zn = cp.tile([P, B], F32, tag="zn" + str(relu))
        nc.vector.tensor_scalar(out=zn[:nparts], in0=zd[:nparts],
                                scalar1=s[:nparts, :1], scalar2=tsh[:nparts, :1],
                                op0=OP.mult, op1=OP.add)
        if relu:
            nc.vector.tensor_scalar(out=zn[:nparts], in0=zn[:nparts],
                                    scalar1=0.0, scalar2=None, op0=OP.max)
        return zn

    with tile.TileContext(nc) as tc:
        with tc.tile_pool(name="c", bufs=1) as cp, \
             tc.tile_pool(name="ps", bufs=1, space="PSUM") as pp:
            zp_s = cp.tile([P, NC * B], F32)
            nc.sync.dma_start(out=zp_s[:], in_=zparts[:])
            ic_s = cp.tile([P, B], F32)
            nc.sync.dma_start(out=ic_s[:], in_=invcnt[:])
            gb_s = cp.tile([P, 1], F32)
            nc.sync.dma_start(out=gb_s[:], in_=gb[:])
            w_s = {}
            for n in names:
                w_s[n] = cp.tile([P, P], F32, tag=n, name=n)
                nc.sync.dma_start(out=w_s[n][:], in_=wts[n][:])
            d3w_s = cp.tile([P, 40], F32)
            nc.sync.dma_start(out=d3w_s[:], in_=d3wT[:])
            v_s = {}
            for n, t_ in vecs.items():
                rows = t_.shape[0]
                v_s[n] = cp.tile([rows, 1], F32, tag="v" + n, name="v" + n)
                nc.sync.dma_start(out=v_s[n][:], in_=t_[:])
            io_s = cp.tile([B, 40], F32)
            nc.sync.dma_start(out=io_s[:], in_=iota40[:])
            id_s = cp.tile([P, P], F32)
            nc.sync.dma_start(out=id_s[:], in_=ident[:])

            zt = cp.tile([P, B], F32)
            nc.vector.tensor_tensor(out=zt[:], in0=zp_s[:, 0:B],
                                    in1=zp_s[:, B:2 * B], op=OP.add)
            for i in range(2, NC):
                nc.vector.tensor_tensor(out=zt[:], in0=zt[:],
                                        in1=zp_s[:, i * B:(i + 1) * B], op=OP.add)
            nc.vector.tensor_tensor(out=zt[:], in0=zt[:], in1=ic_s[:], op=OP.mult)
            nc.vector.tensor_scalar(out=zt[:], in0=zt[:], scalar1=gb_s[:, :1],
                                    scalar2=None, op0=OP.add)

            # d1 + bn1 + relu (two partition-halves of the 256 channels)
            zn1 = []
            for half, (wn, bn_, gn, bbn) in enumerate(
                    [("d1wTa", "d1ba", "bn1ga", "bn1ba"),
                     ("d1wTb", "d1bb", "bn1gb", "bn1bb")]):
                p1 = pp.tile([P, B], F32, space="PSUM", tag="p1" + str(half))
                nc.tensor.matmul(p1[:], lhsT=w_s[wn][:], rhs=zt[:],
                                 start=True, stop=True)
                zd = cp.tile([P, B], F32, tag="zd1" + str(half))
                nc.vector.tensor_scalar(out=zd[:], in0=p1[:],
                                        scalar1=v_s[bn_][:, :1], scalar2=None,
                                        op0=OP.add)
                zn1.append(bn_block(nc, cp, zd, v_s[gn][:, :1], v_s[bbn][:, :1],
                                    True, P))

            # d2 + bn2 + relu
            p2 = pp.tile([P, B], F32, space="PSUM", tag="p2")
            nc.tensor.matmul(p2[:], lhsT=w_s["d2wTa"][:], rhs=zn1[0][:],
                             start=True, stop=False)
            nc.tensor.matmul(p2[:], lhsT=w_s["d2wTb"][:], rhs=zn1[1][:],
                             start=False, stop=True)
            zd2 = cp.tile([P, B], F32)
            nc.vector.tensor_scalar(out=zd2[:], in0=p2[:],
                                    scalar1=v_s["d2b"][:, :1], scalar2=None,
                                    op0=OP.add)
            zn2 = bn_block(nc, cp, zd2, v_s["bn2g"][:, :1], v_s["bn2b"][:, :1],
                           True, P)

            # d3 + bn3
            p3 = pp.tile([40, B], F32, space="PSUM", tag="p3")
            nc.tensor.matmul(p3[:], lhsT=d3w_s[:], rhs=zn2[:],
                             start=True, stop=True)
            zd3 = cp.tile([40, B], F32)
            nc.vector.tensor_scalar(out=zd3[:], in0=p3[:],
                                    scalar1=v_s["d3b"][:, :1], scalar2=None,
                                    op0=OP.add)
            zn3 = bn_block(nc, cp, zd3, v_s["bn3g"][:, :1], v_s["bn3b"][:, :1],
                           False, 40)

            # transpose -> [64, 40], store z, argmax
            ptr = pp.tile([B, 40], F32, space="PSUM", tag="ptr")
            nc.tensor.transpose(out=ptr[:], in_=zn3[:40, :], identity=id_s[:40, :40])
            zf = cp.tile([B, 40], F32)
            nc.vector.tensor_copy(out=zf[:], in_=ptr[:])
            nc.sync.dma_start(out=zout[:], in_=zf[:])
            rmax = cp.tile([B, 1], F32)
            nc.vector.tensor_reduce(out=rmax[:], in_=zf[:],
                                    axis=mybir.AxisListType.X, op=OP.max)
            eqm = cp.tile([B, 40], F32)
            nc.vector.tensor_scalar(out=eqm[:], in0=zf[:],
                                    scalar1=rmax[:, :1], scalar2=None,
                                    op0=OP.is_ge)
            sel = cp.tile([B, 40], F32)
            nc.vector.tensor_scalar(out=sel[:], in0=eqm[:], scalar1=-1e4,
                                    scalar2=1e4, op0=OP.mult, op1=OP.add)
            nc.vector.tensor_tensor(out=sel[:], in0=sel[:], in1=io_s[:], op=OP.add)
            am = cp.tile([B, 1], F32)
            nc.vector.tensor_reduce(out=am[:], in_=sel[:],
                                    axis=mybir.AxisListType.X, op=OP.min)
            ai = cp.tile([B, 1], I32)
            nc.vector.tensor_copy(out=ai[:], in_=am[:])
            nc.sync.dma_start(out=aout[:], in_=ai[:])
    nc.compile()
    return nc


# ------------------------------------------------------------------- driver

def _program(key, builder, meta):
    sig = (key, meta['NPAD'], meta['SC'], tuple(meta['K_sched']))
    if sig not in _programs:
        _programs[sig] = builder(meta)
    return _programs[sig]


def _run(nc, in_maps, tag=None):
    import os
    import time as _time
    t0 = _time.perf_counter()
    res = run_bass_kernel_spmd(nc, in_maps, core_ids=list(range(NC)))
    t1 = _time.perf_counter()
    if os.environ.get("KERNEL_TRACE"):
        LAST.setdefault('times', {})[tag] = int((t1 - t0) * 1e9)
    return res.results


def kernel(pos, edge_index, batch, s1_wl, s1_wr, s1_b, s2_wl, s2_wr, s2_b,
           g_w, g_asrc, g_adst, g_b, d1_w, d1_b, bn1_g, bn1_b,
           d2_w, d2_b, bn2_g, bn2_b, d3_w, d3_b, bn3_g, bn3_b):
    f32 = np.float32
    pos = np.asarray(pos, f32)
    meta, percore = _prep(pos, edge_index, batch)
    NPAD, T, NT = meta['NPAD'], meta['T'], meta['NT']
    new = meta['new_of_old']

    ident = np.eye(P, dtype=f32)
    iota64 = np.tile(np.arange(B, dtype=f32), (P, 1))
    iota40 = np.tile(np.arange(40, dtype=f32), (B, 1))

    # pos4 table [4, NT] in permuted order (pad cols zero)
    pos4 = np.zeros((4, NT), f32)
    pos4[:3, new] = pos.T
    pos4[3, new] = 1.0

    w1l = np.concatenate([np.asarray(s1_wl, f32).T, np.zeros((1, P), f32)], 0)
    w1r = np.concatenate([np.asarray(s1_wr, f32).T,
                          np.asarray(s1_b, f32)[None, :]], 0)

    L1 = _program("L1", _build_L1, meta)
    in_maps = []
    for c in range(NC):
        pc = percore[c]
        p4rot = _rotate(pos4, c, NPAD)
        in_maps.append(dict(
            pos4r=np.ascontiguousarray(p4rot.T), pos4o=p4rot[:, :NPAD].copy(),
            w1l=w1l, w1r=w1r, idx=pc['idx'],
            invd=pc['invd'], realm=pc['realm'], ident=ident))
    r1 = _run(L1, in_maps, "L1")
    x1T_full = np.concatenate([r1[c]["x1T"] for c in range(NC)], axis=1)
    x1R_full = np.concatenate([r1[c]["x1R"] for c in range(NC)], axis=0)
    LAST['x1T'] = x1T_full

    L2 = _program("L2", _build_L2, meta)
    in_maps = []
    for c in range(NC):
        pc = percore[c]
        in_maps.append(dict(
            x1r=_rotate(x1R_full, c, NPAD, axis=0),
            x1To=x1T_full[:, c * NPAD:(c + 1) * NPAD].copy(),
            w2l=np.asarray(s2_wl, f32).T.copy(),
            w2r=np.asarray(s2_wr, f32).T.copy(),
            s2b=np.asarray(s2_b, f32)[None, :], idx=pc['idx'],
            invd=pc['invd'], realm=pc['realm'], ident=ident))
    r2 = _run(L2, in_maps, "L2")
    x2T_full = np.concatenate([r2[c]["x2T"] for c in range(NC)], axis=1)
    LAST['x2T'] = x2T_full

    # W_ext [256, 260]
    g_w = np.asarray(g_w, f32)
    H, C = 2, 128
    w_asrc = np.zeros((256, H), f32)
    w_adst = np.zeros((256, H), f32)
    ga, gd = np.asarray(g_asrc, f32), np.asarray(g_adst, f32)
    for h in range(H):
        w_asrc[:, h] = g_w[h * C:(h + 1) * C, :].T @ ga[h]
        w_adst[:, h] = g_w[h * C:(h + 1) * C, :].T @ gd[h]
    W_ext = np.concatenate([g_w.T, w_asrc, w_adst], axis=1)  # [256, 260]

    L3 = _program("L3", _build_L3, meta)
    in_maps = []
    for c in range(NC):
        pc = percore[c]
        in_maps.append(dict(
            x1T=_rotate(x1T_full, c, NPAD), x2T=_rotate(x2T_full, c, NPAD),
            w3a=W_ext[:128].copy(), w3b=W_ext[128:].copy(),
            idx=pc['idx'], mask=pc['mask'], invdsl=pc['invdsl'],
            batchf=pc['batchf'], iota64=iota64))
    r3 = _run(L3, in_maps, "L3")
    zparts = np.concatenate([r3[c]["zT"] for c in range(NC)], axis=1)
    LAST['zparts'] = zparts

    L4 = _program("L4", _build_L4, meta)
    d1_w = np.asarray(d1_w, f32)
    d2_w = np.asarray(d2_w, f32)
    d3_w = np.asarray(d3_w, f32)
    col = lambda a: np.asarray(a, f32).reshape(-1, 1)
    in_map = dict(
        zparts=zparts, invcnt=np.tile(meta['inv_cnt'][None, :], (P, 1)),
        gb=col(g_b),
        d1wTa=d1_w.T[:, :128].copy(), d1wTb=d1_w.T[:, 128:].copy(),
        d2wTa=d2_w[:, :128].T.copy(), d2wTb=d2_w[:, 128:].T.copy(),
        d3wT=d3_w.T.copy(),
        d1ba=col(d1_b[:128]), d1bb=col(d1_b[128:]),
        bn1ga=col(bn1_g[:128]), bn1gb=col(bn1_g[128:]),
        bn1ba=col(bn1_b[:128]), bn1bb=col(bn1_b[128:]),
        d2b=col(d2_b), bn2g=col(bn2_g), bn2b=col(bn2_b),
        d3b=col(d3_b), bn3g=col(bn3_g), bn3b=col(bn3_b),
        iota40=iota40, ident=ident)
    r4 = _run(L4, [in_map] * NC, "L4")
    z = r4[0]["zout"]
    amax = r4[0]["aout"].reshape(B).astype(np.int32)
    if os.environ.get("KERNEL_PREDICT"):
        try:
            from concourse.timeline_sim import TimelineSim
            tot = 0.0
            for key in ["L1", "L2", "L3", "L4"]:
                sig = (key, meta['NPAD'], meta['SC'], tuple(meta['K_sched']))
                tot += TimelineSim(_programs[sig], trace=False).simulate()
            LAST['predicted_ns'] = int(tot)
        except Exception:
            pass
    return z, amax
